# revision 17
# baseline (speedup 1.0000x reference)
"""Trainium2 Bass kernel for nn_BasicBlock (WeightNet/CondConv-style block).

Data parallel over batch: 32 samples -> 8 cores x 4 samples.

Conv strategy: 1D Winograd F(2,3) along W (M-form), 1.5x fewer PE
cycles than direct conv. Even/odd column planes (host-prepadded, packed
[o-plane, e-plane] per chunk) -> 4 V planes per row-strip via DVE adds.
Generated weights (k-space combine as baseline, split DVE/Pool) get a
cheap 1D U-transform; U2 is built NEGATED so the whole Y-stage runs as:
one ACT evacuation of all 4 M psum planes, two merged 2-lane DVE ops,
one merged relu(+bias) ACT into the next conv's input planes. BN scale
and the 0.5 of the F(2,3) G-matrix are folded into the host basis.
"""

import sys

sys.path.insert(0, "/opt/trn_rl_repo")

import numpy as np
import ml_dtypes

import concourse.bass as bass
import concourse.tile as tile
from concourse import bacc, mybir
from concourse import bass_utils

F32 = mybir.dt.float32
BF16 = mybir.dt.bfloat16
AF = mybir.ActivationFunctionType
ALU = mybir.AluOpType

B, C, H, W = 32, 256, 56, 56
NCORES = 8
BL = B // NCORES          # samples per core
RP = H + 2                # padded rows: 58
TC = W // 2               # tile cols: 28
PW = TC + 2               # plane width: 30
NT, TR = 7, 8             # row-groups
NG = TR * TC              # 224 cols per M plane slice
EPS = 1e-5


def build_program():
    nc = bacc.Bacc("TRN2", target_bir_lowering=False, debug=False,
                   num_devices=NCORES)

    # host-prepadded planes: [s, cc, 128, (o-plane, e-plane), RP, PW]
    xeo4 = nc.dram_tensor("xeo4", [BL, 2, 128, 2, RP, PW], BF16,
                          kind="ExternalInput").ap()
    # out: [s, occ, 128, (even-cols, odd-cols), H, TC]
    outd = nc.dram_tensor("outd", [BL, 2, 128, 2, H, TC], BF16,
                          kind="ExternalOutput").ap()
    rwT = nc.dram_tensor("rwT", [2, 128, 16], F32, kind="ExternalInput").ap()
    rb = nc.dram_tensor("rb", [16, 1], F32, kind="ExternalInput").ap()
    fc1wT = [nc.dram_tensor(f"fc1wT{n}", [16, 4096], BF16,
                            kind="ExternalInput").ap() for n in (1, 2)]
    fc1b = [nc.dram_tensor(f"fc1b{n}", [128, 32], F32,
                           kind="ExternalInput").ap() for n in (1, 2)]
    w2p = [nc.dram_tensor(f"w2p{n}", [2, 128, 4 * 9 * 256], BF16,
                          kind="ExternalInput").ap() for n in (1, 2)]
    bnb = [nc.dram_tensor(f"bnb{n}", [2, 128, 1], F32,
                          kind="ExternalInput").ap() for n in (1, 2)]

    with tile.TileContext(nc) as tc:
        build_body(tc, xeo4, outd, rwT, rb, fc1wT, fc1b, w2p, bnb)

    nc.compile()
    return nc


def build_body(tc, xeo4, outd, rwT, rb, fc1wT, fc1b, w2p, bnb):
    nc = tc.nc
    from contextlib import ExitStack
    ctx = ExitStack()

    cpool = ctx.enter_context(tc.tile_pool(name="consts", bufs=1))
    xeo_p = ctx.enter_context(tc.tile_pool(name="xeo", bufs=2))
    twS_p = ctx.enter_context(tc.tile_pool(name="twS", bufs=3))
    wg_p = ctx.enter_context(tc.tile_pool(name="wgen", bufs=1))
    wtmp_p = ctx.enter_context(tc.tile_pool(name="wtmp", bufs=2))
    small_p = ctx.enter_context(tc.tile_pool(name="small", bufs=2))
    stage_p = ctx.enter_context(tc.tile_pool(name="stage", bufs=2))
    avlin_p = ctx.enter_context(tc.tile_pool(name="avlinp", bufs=1))
    aexp_p = ctx.enter_context(tc.tile_pool(name="aexp", bufs=1))
    psum_p = ctx.enter_context(tc.tile_pool(name="psum", bufs=2, space="PSUM"))
    psmall_p = ctx.enter_context(tc.tile_pool(name="psmall", bufs=1,
                                              space="PSUM"))
    dram_p = ctx.enter_context(tc.tile_pool(name="dscratch", bufs=2,
                                            space="DRAM"))

    # ---- resident constants ----
    w2sb = []   # [wn][cc][i] -> [128, 2304] bf16 (k-major: 9k x 256oc)
    for n in range(2):
        per = []
        for c in range(2):
            per.append([cpool.tile([128, 2304], BF16, tag=f"w2sb{n}{c}{i}",
                                   name=f"w2sb{n}{c}{i}")
                        for i in range(4)])
        w2sb.append(per)

    def load_w2sb(n):
        for c in range(2):
            for i in range(4):
                nc.sync.dma_start(w2sb[n][c][i][:],
                                  w2p[n][c][:, 2304 * i:2304 * (i + 1)])

    rwT_sb = []
    for c in range(2):
        t = cpool.tile([128, 16], F32, tag=f"rwT{c}")
        nc.sync.dma_start(t[:], rwT[c])
        rwT_sb.append(t)
    rb_sb = cpool.tile([16, 1], F32, tag="rb")
    nc.sync.dma_start(rb_sb[:], rb)
    fc1b_sb, bnb_sb = [], []
    fc1w_t = cpool.tile([16, 4096], BF16, tag="fc1w")
    for n in range(2):
        t = cpool.tile([128, 32], F32, tag=f"fc1b{n}")
        if n == 0:
            nc.sync.dma_start(t[:], fc1b[n])
        fc1b_sb.append(t)
        tb = [cpool.tile([128, 1], F32, tag=f"bnb{n}{c}", name=f"bnbt{n}{c}")
              for c in range(2)]
        bnb_sb.append(tb)

    def load_deferred_consts():
        nc.sync.dma_start(fc1b_sb[1][:], fc1b[1])
        for n in range(2):
            for c in range(2):
                nc.sync.dma_start(bnb_sb[n][c][:], bnb[n][c])

    gap16 = cpool.tile([16, BL], BF16, tag="gap16")
    garb = cpool.tile([128, RP * PW], BF16, tag="garb")
    ones_sb = cpool.tile([1, 64], BF16, tag="ones")
    nc.gpsimd.memset(ones_sb[:], 1.0)

    # conv1 output planes (= conv2 input planes): [128, (o,e), RP, PW]
    # zeroed once; sinks only ever write rows 1..56, cols 1..28
    yeo = [cpool.tile([128, 2, RP, PW], BF16, tag=f"yeo{c}", name=f"yeo{c}")
           for c in range(2)]
    for c in range(2):
        nc.gpsimd.memset(yeo[c][:], 0.0)

    # ---- weight generation ----
    def gen_weights_a(wn, s):
        """sigmoid(fc1(gap)) -> partition-broadcast coefficient tiles."""
        nc.sync.dma_start(fc1w_t[:], fc1wT[wn])
        aps = psmall_p.tile([128, 32], F32, tag="avec_ps")
        for j in range(32):
            nc.tensor.matmul(aps[:, j:j + 1],
                             fc1w_t[:, 128 * j:128 * (j + 1)],
                             gap16[:, s:s + 1],
                             start=True, stop=True)
        avt = small_p.tile([128, 32], F32, tag="avtmp")
        nc.vector.tensor_add(avt[:], aps[:], fc1b_sb[wn][:])
        avec = small_p.tile([128, 32], BF16, tag="avec")
        nc.scalar.activation(avec[:], avt[:], AF.Sigmoid)
        avd = dram_p.tile([4096], BF16, tag="avd")
        nc.sync.dma_start(avd[:].rearrange("(j p) -> p j", p=128), avec[:])
        avlin = avlin_p.tile([1, 4096], BF16, tag="avlin")
        nc.sync.dma_start(avlin[:], avd[:].unsqueeze(0))
        avr = avlin[:].rearrange("o (co r) -> o co r", r=16)
        aexp = []
        for c in range(2):
            t = aexp_p.tile([128, 4 * 256], BF16, tag=f"aexp{c}")
            for half in range(2):
                aps2 = psmall_p.tile([128, 2 * 256], F32, tag="aexp_ps")
                for h in range(2):
                    for ii in range(2):
                        i = 2 * half + ii
                        m = 4 * (2 * c + h) + i
                        rhs = avr[:, :, m:m + 1].rearrange("o co r -> o (co r)")
                        nc.tensor.matmul(
                            aps2[64 * h:64 * (h + 1), 256 * ii:256 * (ii + 1)],
                            ones_sb[:], rhs, start=True, stop=True)
                nc.scalar.copy(t[:, 512 * half:512 * (half + 1)], aps2[:])
            aexp.append(t)
        return aexp

    def gen_weights_b(wn, aexp):
        """combine 4 basis tensors -> W [128, 9, 256] then 1D U-transform.

        u12[:, kh, 0] = 0.5(W0+W2) + W1h   (wpos1 stationary)
        u12[:, kh, 1] = W1h - 0.5(W0+W2)   (NEGATED wpos2 stationary)
        W1h pre-halved on host. Combine engine alternates DVE/Pool by cc.
        """
        res = []
        for c in range(2):
            eng = nc.vector if c == 0 else nc.gpsimd
            t = wg_p.tile([128, 9, 256], BF16, tag=f"wg{wn}{c}",
                          name=f"wg{wn}{c}")

            def abid(i):
                return (aexp[c][:, 256 * i:256 * (i + 1)].unsqueeze(1)
                        .broadcast_to([128, 9, 256]))

            def k3(ap2d):
                return ap2d.rearrange("p (k co) -> p k co", k=9)

            eng.tensor_mul(t[:], k3(w2sb[wn][c][0][:]), abid(0))
            for i in range(1, 4):
                tmp = wtmp_p.tile([128, 9, 256], BF16, tag=f"wtmp{c}", bufs=1)
                eng.tensor_mul(tmp[:], k3(w2sb[wn][c][i][:]), abid(i))
                eng.tensor_add(t[:], t[:], tmp[:])
            u12 = wg_p.tile([128, 3, 2, 256], BF16, tag=f"u12{wn}{c}",
                            name=f"u12{wn}{c}")
            for kh in range(3):
                uu = wtmp_p.tile([128, 256], BF16, tag="utmp")
                nc.vector.tensor_add(uu[:], t[:, 3 * kh, :],
                                     t[:, 3 * kh + 2, :])
                nc.vector.tensor_scalar_mul(uu[:], uu[:], 0.5)
                nc.vector.tensor_add(u12[:, kh, 0, :], uu[:],
                                     t[:, 3 * kh + 1, :])
                nc.vector.tensor_sub(u12[:, kh, 1, :],
                                     t[:, 3 * kh + 1, :], uu[:])
            res.append((t, u12))
        return res

    def gen_weights(wn, s):
        return gen_weights_b(wn, gen_weights_a(wn, s))

    def stat(wu, cc, kh, wpos, occ):
        t, u12 = wu[cc]
        if wpos == 0:
            return t[:, 3 * kh + 0, 128 * occ:128 * occ + 128]
        if wpos == 3:
            return t[:, 3 * kh + 2, 128 * occ:128 * occ + 128]
        return u12[:, kh, wpos - 1, 128 * occ:128 * occ + 128]

    # ---- x loading + gap ----
    def load_x(s):
        planes = []
        gacc = []
        for c in range(2):
            te = xeo_p.tile([128, 2, RP, PW], BF16, tag=f"xeo{c}",
                            name=f"xeo{c}")
            nc.sync.dma_start(te[:], xeo4[s, c])
            ga = []
            for a in range(2):
                g = small_p.tile([128, 1], F32, tag=f"gacc{c}{a}",
                                 name=f"gacc{c}{a}")
                nc.scalar.activation(
                    garb[:], te[:, a].rearrange("p h w -> p (h w)"),
                    AF.Copy, accum_out=g[:])
                ga.append(g)
            gs = small_p.tile([128, 1], F32, tag=f"gsum{c}", name=f"gsum{c}")
            nc.vector.tensor_add(gs[:], ga[0][:], ga[1][:])
            gacc.append(gs)
            planes.append(te)
        gps = psmall_p.tile([16, 1], F32, tag="gap_ps")
        for c in range(2):
            nc.tensor.matmul(gps[:], rwT_sb[c][:], gacc[c][:],
                             start=(c == 0), stop=(c == 1))
        nc.scalar.activation(gap16[:, s:s + 1], gps[:], AF.Identity,
                             bias=rb_sb[:], scale=1.0)
        return planes

    # ---- conv: per-t-group V strips + matmuls + sink ----
    # V planes (from o-plane ol/orr, e-plane el/er):
    #   V0 = el - er; V1 = ol + er; V2 = er - ol; V3 = ol - orr
    def conv(wu, planes, sink):
        for t in range(NT):
            r0 = TR * t
            tws = []
            for c in range(2):
                st = twS_p.tile([128, 4, TR + 2, TC], BF16, tag=f"tws{c}",
                                name=f"tws{c}")
                e = planes[c][:, 1, r0:r0 + TR + 2, :]
                o = planes[c][:, 0, r0:r0 + TR + 2, :]
                el = e[:, :, 0:TC]
                er = e[:, :, 1:TC + 1]
                ol = o[:, :, 1:TC + 1]
                orr = o[:, :, 2:TC + 2]
                nc.vector.tensor_sub(st[:, 0], el, er)
                nc.vector.tensor_add(st[:, 1], ol, er)
                nc.vector.tensor_sub(st[:, 2], er, ol)
                nc.vector.tensor_sub(st[:, 3], ol, orr)
                tws.append(st)
            for occ in range(2):
                ps = psum_p.tile([128, 4, 256], F32, tag="cps")
                for wpos in range(4):
                    for cc in range(2):
                        for kh in range(3):
                            nc.tensor.matmul(
                                ps[:, wpos, 0:NG],
                                stat(wu, cc, kh, wpos, occ),
                                tws[cc][:, wpos, kh:kh + TR, :],
                                start=(cc == 0 and kh == 0),
                                stop=(cc == 1 and kh == 2))
                sink(occ, t, ps)

    # Y-stage: with M2' = -M2 (negated U2 stationary):
    #   [e1,o1] = m[(0,2)] + bcast(m1);  [e2,o2] = [e1,o1] - m[(2,3)]
    #   e2 = M0+M1+M2 (even outputs), o2 = M1-M2-M3 (odd outputs)
    def ystage(ps):
        m = stage_p.tile([128, 4, NG], BF16, tag="mev")
        nc.scalar.copy(m[:], ps[:, :, 0:NG])
        m02 = m[:].rearrange("p (j two) n -> p j two n", two=2)[:, :, 0]
        eo1 = stage_p.tile([128, 2, NG], BF16, tag="eo1", bufs=1)
        nc.vector.tensor_add(
            eo1[:], m02, m[:, 1].unsqueeze(1).broadcast_to([128, 2, NG]))
        eo2 = stage_p.tile([128, 2, NG], BF16, tag="eo2")
        nc.vector.tensor_sub(eo2[:], eo1[:], m[:, 2:4])
        return eo2

    def sink1(occ, t, ps):
        r0 = TR * t + 1
        eo2 = ystage(ps)
        # even outputs -> o-plane, odd outputs -> e-plane, cols 1..28
        nc.scalar.activation(
            yeo[occ][:, :, r0:r0 + TR, 1:TC + 1],
            eo2[:].rearrange("p a (h w) -> p a h w", h=TR),
            AF.Relu, bias=bnb_sb[0][occ][:], scale=1.0)

    def make_sink2(s, xplanes):
        def sink2(occ, t, ps):
            r0 = TR * t + 1
            eo2 = ystage(ps)
            rx = xplanes[occ][:, :, r0:r0 + TR, 1:TC + 1]
            eo3 = stage_p.tile([128, 2, TR, TC], BF16, tag="eo3")
            nc.vector.tensor_add(
                eo3[:], eo2[:].rearrange("p a (h w) -> p a h w", h=TR), rx)
            os_ = stage_p.tile([128, 2, TR, TC], BF16, tag="ostg")
            nc.scalar.activation(os_[:], eo3[:], AF.Relu,
                                 bias=bnb_sb[1][occ][:], scale=1.0)
            nc.sync.dma_start(outd[s, occ][:, :, TR * t:TR * t + TR, :],
                              os_[:])
        return sink2

    # ---- main pipeline ----
    xp = load_x(0)
    ax0 = gen_weights_a(0, 0)
    load_w2sb(0)
    w1 = gen_weights_b(0, ax0)
    load_deferred_consts()
    load_w2sb(1)

    for s in range(BL):
        w2 = gen_weights(1, s)
        if s + 1 < BL:
            xp_n = load_x(s + 1)
            w1_n = gen_weights(0, s + 1)

        conv(w1, xp, sink1)
        conv(w2, yeo, make_sink2(s, xp))

        if s + 1 < BL:
            xp, w1 = xp_n, w1_n

    ctx.close()


_NC_CACHE = {}


def get_program():
    if "nc" not in _NC_CACHE:
        _NC_CACHE["nc"] = build_program()
    return _NC_CACHE["nc"]


def prep_inputs(inputs):
    x = np.asarray(inputs["x"], np.float32)
    f32 = lambda a: np.ascontiguousarray(np.asarray(a, np.float32))
    bf = lambda a: np.ascontiguousarray(
        np.asarray(a, np.float32).astype(ml_dtypes.bfloat16))

    def bn_fold(g, b, m, v):
        sc = np.asarray(g, np.float32) / np.sqrt(np.asarray(v, np.float32) + EPS)
        bia = np.asarray(b, np.float32) - np.asarray(m, np.float32) * sc
        return sc, f32(bia.reshape(2, 128, 1))

    def pack_w2(fc2_w, bn_sc):
        w2_ = np.asarray(fc2_w, np.float32).reshape(256, 4, 64, 9, 4)
        w2_ = w2_ * bn_sc[:, None, None, None, None]   # fold bn scale (per oc)
        w2_[:, :, :, 1::3, :] *= 0.5                   # pre-halve kw=1 taps
        w2h = w2_.transpose(4, 3, 1, 2, 0).reshape(4, 9, 256, 256)
        return bf(w2h.transpose(2, 0, 1, 3).reshape(2, 128, 4 * 9 * 256))

    s1, b1 = bn_fold(inputs["bn1_g"], inputs["bn1_b"],
                     inputs["bn1_m"], inputs["bn1_v"])
    s2, b2 = bn_fold(inputs["bn2_g"], inputs["bn2_b"],
                     inputs["bn2_m"], inputs["bn2_v"])

    NPIX = H * W
    base = {
        "rwT": f32((np.asarray(inputs["reduce_w"], np.float32).T / NPIX)
                   .reshape(2, 128, 16)),
        "rb": f32(np.asarray(inputs["reduce_b"]).reshape(16, 1)),
        "fc1wT1": bf(np.asarray(inputs["w1_fc1_w"]).T),
        "fc1wT2": bf(np.asarray(inputs["w2_fc1_w"]).T),
        "fc1b1": f32(np.asarray(inputs["w1_fc1_b"]).reshape(32, 128).T),
        "fc1b2": f32(np.asarray(inputs["w2_fc1_b"]).reshape(32, 128).T),
        "w2p1": pack_w2(inputs["w1_fc2_w"], s1),
        "w2p2": pack_w2(inputs["w2_fc2_w"], s2),
        "bnb1": b1,
        "bnb2": b2,
    }

    # host-prepadded planes: o-plane[j] = xpad[2j-1] (x even cols, at 1..28),
    # e-plane[j] = xpad[2j] (x odd cols at 1..28; col 0 = xpad[0] = 0)
    xb = x.astype(ml_dtypes.bfloat16)
    xeo = np.zeros((B, C, 2, RP, PW), ml_dtypes.bfloat16)
    xeo[:, :, 0, 1:RP - 1, 1:TC + 1] = xb[:, :, :, 0::2]
    xeo[:, :, 1, 1:RP - 1, 1:TC + 1] = xb[:, :, :, 1::2]

    in_maps = []
    for i in range(NCORES):
        m = dict(base)
        m["xeo4"] = np.ascontiguousarray(
            xeo[i * BL:(i + 1) * BL].reshape(BL, 2, 128, 2, RP, PW))
        in_maps.append(m)
    return in_maps


def unpack_outputs(results):
    outs = []
    for r in results:
        od = np.asarray(r["outd"], ml_dtypes.bfloat16).astype(np.float32)
        out = np.zeros((BL, 2, 128, H, W), np.float32)
        out[..., 0::2] = od[:, :, :, 0]
        out[..., 1::2] = od[:, :, :, 1]
        outs.append(out.reshape(BL, C, H, W))
    return np.concatenate(outs, axis=0)


def kernel(**inputs):
    in_maps = prep_inputs(inputs)
    nc = get_program()
    res = bass_utils.run_bass_kernel_spmd(nc, in_maps,
                                          core_ids=list(range(NCORES)))
    return unpack_outputs(res.results)


# revision 18
# speedup vs baseline: 1.1264x; 1.1264x over previous
"""Trainium2 Bass kernel for nn_BasicBlock (WeightNet/CondConv-style block).

Data parallel over batch: 32 samples -> 8 cores x 4 samples.

Conv strategy: 1D Winograd F(2,3) along W (M-form), 1.5x fewer PE
cycles than direct conv. Even/odd column planes (host-prepadded, packed
[o-plane, e-plane] per chunk) -> 4 V planes per row-strip via DVE adds.
Generated weights (k-space combine as baseline, split DVE/Pool) get a
cheap 1D U-transform; U2 is built NEGATED so the whole Y-stage runs as:
one ACT evacuation of all 4 M psum planes, two merged 2-lane DVE ops,
one merged relu(+bias) ACT into the next conv's input planes. BN scale
and the 0.5 of the F(2,3) G-matrix are folded into the host basis.
"""

import sys

sys.path.insert(0, "/opt/trn_rl_repo")

import numpy as np
import ml_dtypes

import concourse.bass as bass
import concourse.tile as tile
from concourse import bacc, mybir
from concourse import bass_utils

F32 = mybir.dt.float32
BF16 = mybir.dt.bfloat16
AF = mybir.ActivationFunctionType
ALU = mybir.AluOpType

B, C, H, W = 32, 256, 56, 56
NCORES = 8
BL = B // NCORES          # samples per core
RP = H + 2                # padded rows: 58
TC = W // 2               # tile cols: 28
PW = TC + 2               # plane width: 30
NT, TR = 7, 8             # row-groups
NG = TR * TC              # 224 cols per M plane slice
EPS = 1e-5


def build_program():
    nc = bacc.Bacc("TRN2", target_bir_lowering=False, debug=False,
                   num_devices=NCORES)

    # host-prepadded planes: [s, cc, 128, (o-plane, e-plane), RP, PW]
    xeo4 = nc.dram_tensor("xeo4", [BL, 2, 128, 2, RP, PW], BF16,
                          kind="ExternalInput").ap()
    # out: [s, occ, 128, (even-cols, odd-cols), H, TC]
    outd = nc.dram_tensor("outd", [BL, 2, 128, 2, H, TC], BF16,
                          kind="ExternalOutput").ap()
    rwT = nc.dram_tensor("rwT", [2, 128, 16], F32, kind="ExternalInput").ap()
    rb = nc.dram_tensor("rb", [16, 1], F32, kind="ExternalInput").ap()
    fc1wT = [nc.dram_tensor(f"fc1wT{n}", [16, 4096], BF16,
                            kind="ExternalInput").ap() for n in (1, 2)]
    fc1b = [nc.dram_tensor(f"fc1b{n}", [128, 32], F32,
                           kind="ExternalInput").ap() for n in (1, 2)]
    w2p = [nc.dram_tensor(f"w2p{n}", [2, 128, 4 * 9 * 256], BF16,
                          kind="ExternalInput").ap() for n in (1, 2)]
    bnb = [nc.dram_tensor(f"bnb{n}", [2, 128, 1], F32,
                          kind="ExternalInput").ap() for n in (1, 2)]

    with tile.TileContext(nc) as tc:
        build_body(tc, xeo4, outd, rwT, rb, fc1wT, fc1b, w2p, bnb)

    nc.compile()
    return nc


def build_body(tc, xeo4, outd, rwT, rb, fc1wT, fc1b, w2p, bnb):
    nc = tc.nc
    from contextlib import ExitStack
    ctx = ExitStack()

    cpool = ctx.enter_context(tc.tile_pool(name="consts", bufs=1))
    xeo_p = ctx.enter_context(tc.tile_pool(name="xeo", bufs=2))
    twS_p = ctx.enter_context(tc.tile_pool(name="twS", bufs=3))
    wg_p = ctx.enter_context(tc.tile_pool(name="wgen", bufs=1))
    wtmp_p = ctx.enter_context(tc.tile_pool(name="wtmp", bufs=2))
    small_p = ctx.enter_context(tc.tile_pool(name="small", bufs=2))
    stage_p = ctx.enter_context(tc.tile_pool(name="stage", bufs=2))
    avlin_p = ctx.enter_context(tc.tile_pool(name="avlinp", bufs=1))
    aexp_p = ctx.enter_context(tc.tile_pool(name="aexp", bufs=1))
    psum_p = ctx.enter_context(tc.tile_pool(name="psum", bufs=3, space="PSUM"))
    psmall_p = ctx.enter_context(tc.tile_pool(name="psmall", bufs=1,
                                              space="PSUM"))
    dram_p = ctx.enter_context(tc.tile_pool(name="dscratch", bufs=2,
                                            space="DRAM"))

    # ---- resident constants ----
    w2sb = []   # [wn][cc][i] -> [128, 2304] bf16 (k-major: 9k x 256oc)
    for n in range(2):
        per = []
        for c in range(2):
            per.append([cpool.tile([128, 2304], BF16, tag=f"w2sb{n}{c}{i}",
                                   name=f"w2sb{n}{c}{i}")
                        for i in range(4)])
        w2sb.append(per)

    def load_w2sb(n):
        for c in range(2):
            for i in range(4):
                nc.sync.dma_start(w2sb[n][c][i][:],
                                  w2p[n][c][:, 2304 * i:2304 * (i + 1)])

    rwT_sb = []
    for c in range(2):
        t = cpool.tile([128, 16], F32, tag=f"rwT{c}")
        nc.sync.dma_start(t[:], rwT[c])
        rwT_sb.append(t)
    rb_sb = cpool.tile([16, 1], F32, tag="rb")
    nc.sync.dma_start(rb_sb[:], rb)
    fc1b_sb, bnb_sb = [], []
    fc1w_t = cpool.tile([16, 4096], BF16, tag="fc1w")
    for n in range(2):
        t = cpool.tile([128, 32], F32, tag=f"fc1b{n}")
        if n == 0:
            nc.sync.dma_start(t[:], fc1b[n])
        fc1b_sb.append(t)
        tb = [cpool.tile([128, 1], F32, tag=f"bnb{n}{c}", name=f"bnbt{n}{c}")
              for c in range(2)]
        bnb_sb.append(tb)

    def load_deferred_consts():
        nc.sync.dma_start(fc1b_sb[1][:], fc1b[1])
        for n in range(2):
            for c in range(2):
                nc.sync.dma_start(bnb_sb[n][c][:], bnb[n][c])

    gap16 = cpool.tile([16, BL], BF16, tag="gap16")
    garb = cpool.tile([128, RP * PW], BF16, tag="garb")
    ones_sb = cpool.tile([1, 64], BF16, tag="ones")
    nc.gpsimd.memset(ones_sb[:], 1.0)

    # conv1 output planes (= conv2 input planes): [128, (o,e), RP, PW]
    # zeroed once; sinks only ever write rows 1..56, cols 1..28
    yeo = [cpool.tile([128, 2, RP, PW], BF16, tag=f"yeo{c}", name=f"yeo{c}")
           for c in range(2)]
    for c in range(2):
        nc.gpsimd.memset(yeo[c][:], 0.0)

    # ---- weight generation ----
    def gen_weights_a(wn, s):
        """sigmoid(fc1(gap)) -> partition-broadcast coefficient tiles."""
        nc.sync.dma_start(fc1w_t[:], fc1wT[wn])
        apsg = psmall_p.tile([128, 33], F32, tag="avec_ps")
        aps = apsg[:, 0:32]
        for j in range(32):
            nc.tensor.matmul(aps[:, j:j + 1],
                             fc1w_t[:, 128 * j:128 * (j + 1)],
                             gap16[:, s:s + 1],
                             start=True, stop=True)
        avt = small_p.tile([128, 32], F32, tag="avtmp")
        nc.vector.tensor_add(avt[:], aps, fc1b_sb[wn][:])
        avec = small_p.tile([128, 32], BF16, tag="avec")
        nc.scalar.activation(avec[:], avt[:], AF.Sigmoid)
        avd = dram_p.tile([4096], BF16, tag="avd")
        nc.sync.dma_start(avd[:].rearrange("(j p) -> p j", p=128), avec[:])
        avlin = avlin_p.tile([1, 4096], BF16, tag="avlin")
        nc.sync.dma_start(avlin[:], avd[:].unsqueeze(0))
        avr = avlin[:].rearrange("o (co r) -> o co r", r=16)
        aexp = []
        for c in range(2):
            t = aexp_p.tile([128, 4 * 256], BF16, tag=f"aexp{c}")
            for half in range(2):
                aps2 = psmall_p.tile([128, 2 * 256], F32, tag="aexp_ps")
                for h in range(2):
                    for ii in range(2):
                        i = 2 * half + ii
                        m = 4 * (2 * c + h) + i
                        rhs = avr[:, :, m:m + 1].rearrange("o co r -> o (co r)")
                        nc.tensor.matmul(
                            aps2[64 * h:64 * (h + 1), 256 * ii:256 * (ii + 1)],
                            ones_sb[:], rhs, start=True, stop=True)
                nc.scalar.copy(t[:, 512 * half:512 * (half + 1)], aps2[:])
            aexp.append(t)
        return aexp

    def gen_weights_b(wn, aexp):
        """combine 4 basis tensors -> W [128, 9, 256] then 1D U-transform.

        u12[:, kh, 0] = 0.5(W0+W2) + W1h   (wpos1 stationary)
        u12[:, kh, 1] = W1h - 0.5(W0+W2)   (NEGATED wpos2 stationary)
        W1h pre-halved on host. wn==0 instances tree-split DVE/Pool.
        """
        res = []
        for c in range(2):
            t = wg_p.tile([128, 9, 256], BF16, tag=f"wg{wn}{c}",
                          name=f"wg{wn}{c}")

            def abid(i):
                return (aexp[c][:, 256 * i:256 * (i + 1)].unsqueeze(1)
                        .broadcast_to([128, 9, 256]))

            def k3(ap2d):
                return ap2d.rearrange("p (k co) -> p k co", k=9)

            if wn == 0:
                # DVE: t = b0*a0 + b1*a1 ; Pool: t23 = b2*a2 + b3*a3
                tmpD = wtmp_p.tile([128, 9, 256], BF16, tag="wtmpD", bufs=1)
                t23 = wtmp_p.tile([128, 9, 256], BF16, tag="t23", bufs=1)
                tmpP = wtmp_p.tile([128, 9, 256], BF16, tag="wtmpP", bufs=1)
                nc.vector.tensor_mul(t[:], k3(w2sb[wn][c][0][:]), abid(0))
                nc.vector.tensor_mul(tmpD[:], k3(w2sb[wn][c][1][:]), abid(1))
                nc.vector.tensor_add(t[:], t[:], tmpD[:])
                nc.gpsimd.tensor_mul(t23[:], k3(w2sb[wn][c][2][:]), abid(2))
                nc.gpsimd.tensor_mul(tmpP[:], k3(w2sb[wn][c][3][:]), abid(3))
                nc.gpsimd.tensor_add(t23[:], t23[:], tmpP[:])
                nc.vector.tensor_add(t[:], t[:], t23[:])
            else:
                nc.vector.tensor_mul(t[:], k3(w2sb[wn][c][0][:]), abid(0))
                for i in range(1, 4):
                    tmp = wtmp_p.tile([128, 9, 256], BF16, tag="wtmpD",
                                      bufs=1)
                    nc.vector.tensor_mul(tmp[:], k3(w2sb[wn][c][i][:]),
                                         abid(i))
                    nc.vector.tensor_add(t[:], t[:], tmp[:])
            u12 = wg_p.tile([128, 3, 2, 256], BF16, tag=f"u12{wn}{c}",
                            name=f"u12{wn}{c}")
            for kh in range(3):
                uu = wtmp_p.tile([128, 256], BF16, tag="utmp")
                nc.vector.tensor_add(uu[:], t[:, 3 * kh, :],
                                     t[:, 3 * kh + 2, :])
                nc.vector.tensor_scalar_mul(uu[:], uu[:], 0.5)
                nc.vector.tensor_add(u12[:, kh, 0, :], uu[:],
                                     t[:, 3 * kh + 1, :])
                nc.vector.tensor_sub(u12[:, kh, 1, :],
                                     t[:, 3 * kh + 1, :], uu[:])
            res.append((t, u12))
        return res

    def gen_weights(wn, s):
        return gen_weights_b(wn, gen_weights_a(wn, s))

    def stat(wu, cc, kh, wpos, occ):
        t, u12 = wu[cc]
        if wpos == 0:
            return t[:, 3 * kh + 0, 128 * occ:128 * occ + 128]
        if wpos == 3:
            return t[:, 3 * kh + 2, 128 * occ:128 * occ + 128]
        return u12[:, kh, wpos - 1, 128 * occ:128 * occ + 128]

    # ---- x loading + gap ----
    def load_x(s):
        planes = []
        gacc = []
        for c in range(2):
            te = xeo_p.tile([128, 2, RP, PW], BF16, tag=f"xeo{c}",
                            name=f"xeo{c}")
            nc.sync.dma_start(te[:], xeo4[s, c])
            ga = []
            for a in range(2):
                g = small_p.tile([128, 1], F32, tag=f"gacc{c}{a}",
                                 name=f"gacc{c}{a}")
                nc.scalar.activation(
                    garb[:], te[:, a].rearrange("p h w -> p (h w)"),
                    AF.Copy, accum_out=g[:])
                ga.append(g)
            gs = small_p.tile([128, 1], F32, tag=f"gsum{c}", name=f"gsum{c}")
            nc.vector.tensor_add(gs[:], ga[0][:], ga[1][:])
            gacc.append(gs)
            planes.append(te)
        apsg = psmall_p.tile([128, 33], F32, tag="avec_ps")
        gps = apsg[0:16, 32:33]
        for c in range(2):
            nc.tensor.matmul(gps, rwT_sb[c][:], gacc[c][:],
                             start=(c == 0), stop=(c == 1))
        nc.scalar.activation(gap16[:, s:s + 1], gps, AF.Identity,
                             bias=rb_sb[:], scale=1.0)
        return planes

    # ---- conv: per-t-group V strips + matmuls + sink ----
    # V planes (from o-plane ol/orr, e-plane el/er):
    #   V0 = el - er; V1 = ol + er; V2 = er - ol; V3 = ol - orr
    def conv(wu, planes, sink):
        for t in range(NT):
            r0 = TR * t
            tws = []
            for c in range(2):
                st = twS_p.tile([128, 4, TR + 2, TC], BF16, tag=f"tws{c}",
                                name=f"tws{c}")
                e = planes[c][:, 1, r0:r0 + TR + 2, :]
                o = planes[c][:, 0, r0:r0 + TR + 2, :]
                el = e[:, :, 0:TC]
                er = e[:, :, 1:TC + 1]
                ol = o[:, :, 1:TC + 1]
                orr = o[:, :, 2:TC + 2]
                nc.vector.tensor_sub(st[:, 0], el, er)
                nc.vector.tensor_add(st[:, 1], ol, er)
                nc.gpsimd.tensor_sub(st[:, 2], er, ol)
                nc.gpsimd.tensor_sub(st[:, 3], ol, orr)
                tws.append(st)
            for occ in range(2):
                ps = psum_p.tile([128, 4, 256], F32, tag="cps")
                for wpos in range(4):
                    for cc in range(2):
                        for kh in range(3):
                            nc.tensor.matmul(
                                ps[:, wpos, 0:NG],
                                stat(wu, cc, kh, wpos, occ),
                                tws[cc][:, wpos, kh:kh + TR, :],
                                start=(cc == 0 and kh == 0),
                                stop=(cc == 1 and kh == 2))
                sink(occ, t, ps)

    # Y-stage: with M2' = -M2 (negated U2 stationary):
    #   [e1,o1] = m[(0,2)] + bcast(m1);  [e2,o2] = [e1,o1] - m[(2,3)]
    #   e2 = M0+M1+M2 (even outputs), o2 = M1-M2-M3 (odd outputs)
    def ystage(ps):
        m = stage_p.tile([128, 4, NG], BF16, tag="mev")
        nc.scalar.copy(m[:], ps[:, :, 0:NG])
        m02 = m[:].rearrange("p (j two) n -> p j two n", two=2)[:, :, 0]
        eo1 = stage_p.tile([128, 2, NG], BF16, tag="eo1", bufs=1)
        nc.vector.tensor_add(
            eo1[:], m02, m[:, 1].unsqueeze(1).broadcast_to([128, 2, NG]))
        eo2 = stage_p.tile([128, 2, NG], BF16, tag="eo2")
        nc.vector.tensor_sub(eo2[:], eo1[:], m[:, 2:4])
        return eo2

    def sink1(occ, t, ps):
        r0 = TR * t + 1
        eo2 = ystage(ps)
        # even outputs -> o-plane, odd outputs -> e-plane, cols 1..28
        nc.scalar.activation(
            yeo[occ][:, :, r0:r0 + TR, 1:TC + 1],
            eo2[:].rearrange("p a (h w) -> p a h w", h=TR),
            AF.Relu, bias=bnb_sb[0][occ][:], scale=1.0)

    def make_sink2(s, xplanes):
        def sink2(occ, t, ps):
            r0 = TR * t + 1
            eo2 = ystage(ps)
            rx = xplanes[occ][:, :, r0:r0 + TR, 1:TC + 1]
            eo3 = stage_p.tile([128, 2, TR, TC], BF16, tag="eo3")
            nc.vector.tensor_add(
                eo3[:], eo2[:].rearrange("p a (h w) -> p a h w", h=TR), rx)
            os_ = stage_p.tile([128, 2, TR, TC], BF16, tag="ostg")
            nc.scalar.activation(os_[:], eo3[:], AF.Relu,
                                 bias=bnb_sb[1][occ][:], scale=1.0)
            nc.sync.dma_start(outd[s, occ][:, :, TR * t:TR * t + TR, :],
                              os_[:])
        return sink2

    # ---- main pipeline ----
    xp = load_x(0)
    ax0 = gen_weights_a(0, 0)
    load_w2sb(0)
    w1 = gen_weights_b(0, ax0)
    load_deferred_consts()
    load_w2sb(1)

    for s in range(BL):
        w2 = gen_weights(1, s)
        if s + 1 < BL:
            xp_n = load_x(s + 1)
            w1_n = gen_weights(0, s + 1)

        conv(w1, xp, sink1)
        conv(w2, yeo, make_sink2(s, xp))

        if s + 1 < BL:
            xp, w1 = xp_n, w1_n

    ctx.close()


_NC_CACHE = {}


def get_program():
    if "nc" not in _NC_CACHE:
        _NC_CACHE["nc"] = build_program()
    return _NC_CACHE["nc"]


def prep_inputs(inputs):
    x = np.asarray(inputs["x"], np.float32)
    f32 = lambda a: np.ascontiguousarray(np.asarray(a, np.float32))
    bf = lambda a: np.ascontiguousarray(
        np.asarray(a, np.float32).astype(ml_dtypes.bfloat16))

    def bn_fold(g, b, m, v):
        sc = np.asarray(g, np.float32) / np.sqrt(np.asarray(v, np.float32) + EPS)
        bia = np.asarray(b, np.float32) - np.asarray(m, np.float32) * sc
        return sc, f32(bia.reshape(2, 128, 1))

    def pack_w2(fc2_w, bn_sc):
        w2_ = np.asarray(fc2_w, np.float32).reshape(256, 4, 64, 9, 4)
        w2_ = w2_ * bn_sc[:, None, None, None, None]   # fold bn scale (per oc)
        w2_[:, :, :, 1::3, :] *= 0.5                   # pre-halve kw=1 taps
        w2h = w2_.transpose(4, 3, 1, 2, 0).reshape(4, 9, 256, 256)
        return bf(w2h.transpose(2, 0, 1, 3).reshape(2, 128, 4 * 9 * 256))

    s1, b1 = bn_fold(inputs["bn1_g"], inputs["bn1_b"],
                     inputs["bn1_m"], inputs["bn1_v"])
    s2, b2 = bn_fold(inputs["bn2_g"], inputs["bn2_b"],
                     inputs["bn2_m"], inputs["bn2_v"])

    NPIX = H * W
    base = {
        "rwT": f32((np.asarray(inputs["reduce_w"], np.float32).T / NPIX)
                   .reshape(2, 128, 16)),
        "rb": f32(np.asarray(inputs["reduce_b"]).reshape(16, 1)),
        "fc1wT1": bf(np.asarray(inputs["w1_fc1_w"]).T),
        "fc1wT2": bf(np.asarray(inputs["w2_fc1_w"]).T),
        "fc1b1": f32(np.asarray(inputs["w1_fc1_b"]).reshape(32, 128).T),
        "fc1b2": f32(np.asarray(inputs["w2_fc1_b"]).reshape(32, 128).T),
        "w2p1": pack_w2(inputs["w1_fc2_w"], s1),
        "w2p2": pack_w2(inputs["w2_fc2_w"], s2),
        "bnb1": b1,
        "bnb2": b2,
    }

    # host-prepadded planes: o-plane[j] = xpad[2j-1] (x even cols, at 1..28),
    # e-plane[j] = xpad[2j] (x odd cols at 1..28; col 0 = xpad[0] = 0)
    xb = x.astype(ml_dtypes.bfloat16)
    xeo = np.zeros((B, C, 2, RP, PW), ml_dtypes.bfloat16)
    xeo[:, :, 0, 1:RP - 1, 1:TC + 1] = xb[:, :, :, 0::2]
    xeo[:, :, 1, 1:RP - 1, 1:TC + 1] = xb[:, :, :, 1::2]

    in_maps = []
    for i in range(NCORES):
        m = dict(base)
        m["xeo4"] = np.ascontiguousarray(
            xeo[i * BL:(i + 1) * BL].reshape(BL, 2, 128, 2, RP, PW))
        in_maps.append(m)
    return in_maps


def unpack_outputs(results):
    outs = []
    for r in results:
        od = np.asarray(r["outd"], ml_dtypes.bfloat16).astype(np.float32)
        out = np.zeros((BL, 2, 128, H, W), np.float32)
        out[..., 0::2] = od[:, :, :, 0]
        out[..., 1::2] = od[:, :, :, 1]
        outs.append(out.reshape(BL, C, H, W))
    return np.concatenate(outs, axis=0)


def kernel(**inputs):
    in_maps = prep_inputs(inputs)
    nc = get_program()
    res = bass_utils.run_bass_kernel_spmd(nc, in_maps,
                                          core_ids=list(range(NCORES)))
    return unpack_outputs(res.results)


# revision 19
# speedup vs baseline: 1.2244x; 1.0871x over previous
"""Trainium2 Bass kernel for nn_BasicBlock (WeightNet/CondConv-style block).

Data parallel over batch: 32 samples -> 8 cores x 4 samples.

Conv strategy: 1D Winograd F(2,3) along W (M-form), 1.5x fewer PE
cycles than direct conv. Even/odd column planes (host-prepadded, packed
[o-plane, e-plane] per chunk) -> 4 V planes per row-strip via DVE adds.
Generated weights (k-space combine as baseline, split DVE/Pool) get a
cheap 1D U-transform; U2 is built NEGATED so the whole Y-stage runs as:
one ACT evacuation of all 4 M psum planes, two merged 2-lane DVE ops,
one merged relu(+bias) ACT into the next conv's input planes. BN scale
and the 0.5 of the F(2,3) G-matrix are folded into the host basis.
"""

import sys

sys.path.insert(0, "/opt/trn_rl_repo")

import numpy as np
import ml_dtypes

import concourse.bass as bass
import concourse.tile as tile
from concourse import bacc, mybir
from concourse import bass_utils

F32 = mybir.dt.float32
BF16 = mybir.dt.bfloat16
AF = mybir.ActivationFunctionType
ALU = mybir.AluOpType

B, C, H, W = 32, 256, 56, 56
NCORES = 8
BL = B // NCORES          # samples per core
RP = H + 2                # padded rows: 58
TC = W // 2               # tile cols: 28
PW = TC + 2               # plane width: 30
NT, TR = 7, 8             # row-groups
NG = TR * TC              # 224 cols per M plane slice
EPS = 1e-5


def build_program():
    nc = bacc.Bacc("TRN2", target_bir_lowering=False, debug=False,
                   num_devices=NCORES)

    # host-prepadded planes: [s, cc, 128, (o-plane, e-plane), RP, PW]
    xeo4 = nc.dram_tensor("xeo4", [BL, 2, 128, 2, RP, PW], BF16,
                          kind="ExternalInput").ap()
    # out: [s, occ, 128, (even-cols, odd-cols), H, TC]
    outd = nc.dram_tensor("outd", [BL, 2, 128, 2, H, TC], BF16,
                          kind="ExternalOutput").ap()
    rwT = nc.dram_tensor("rwT", [2, 128, 16], F32, kind="ExternalInput").ap()
    rb = nc.dram_tensor("rb", [16, 1], F32, kind="ExternalInput").ap()
    fc1wT = [nc.dram_tensor(f"fc1wT{n}", [16, 4096], BF16,
                            kind="ExternalInput").ap() for n in (1, 2)]
    fc1b = [nc.dram_tensor(f"fc1b{n}", [128, 32], F32,
                           kind="ExternalInput").ap() for n in (1, 2)]
    w2p = [nc.dram_tensor(f"w2p{n}", [2, 128, 4 * 9 * 256], BF16,
                          kind="ExternalInput").ap() for n in (1, 2)]
    bnb = [nc.dram_tensor(f"bnb{n}", [2, 128, 1], F32,
                          kind="ExternalInput").ap() for n in (1, 2)]

    with tile.TileContext(nc) as tc:
        build_body(tc, xeo4, outd, rwT, rb, fc1wT, fc1b, w2p, bnb)

    nc.compile()
    return nc


def build_body(tc, xeo4, outd, rwT, rb, fc1wT, fc1b, w2p, bnb):
    nc = tc.nc
    from contextlib import ExitStack
    ctx = ExitStack()

    cpool = ctx.enter_context(tc.tile_pool(name="consts", bufs=1))
    xeo_p = ctx.enter_context(tc.tile_pool(name="xeo", bufs=2))
    twS_p = ctx.enter_context(tc.tile_pool(name="twS", bufs=3))
    wg_p = ctx.enter_context(tc.tile_pool(name="wgen", bufs=1))
    wtmp_p = ctx.enter_context(tc.tile_pool(name="wtmp", bufs=2))
    small_p = ctx.enter_context(tc.tile_pool(name="small", bufs=2))
    stage_p = ctx.enter_context(tc.tile_pool(name="stage", bufs=2))
    avlin_p = ctx.enter_context(tc.tile_pool(name="avlinp", bufs=1))
    aexp_p = ctx.enter_context(tc.tile_pool(name="aexp", bufs=1))
    psum_p = ctx.enter_context(tc.tile_pool(name="psum", bufs=3, space="PSUM"))
    psmall_p = ctx.enter_context(tc.tile_pool(name="psmall", bufs=1,
                                              space="PSUM"))
    dram_p = ctx.enter_context(tc.tile_pool(name="dscratch", bufs=2,
                                            space="DRAM"))

    # ---- resident constants ----
    w2sb = []   # [wn][cc][i] -> [128, 2304] bf16 (k-major: 9k x 256oc)
    for n in range(2):
        per = []
        for c in range(2):
            per.append([cpool.tile([128, 2304], BF16, tag=f"w2sb{n}{c}{i}",
                                   name=f"w2sb{n}{c}{i}")
                        for i in range(4)])
        w2sb.append(per)

    def load_w2sb(n):
        for c in range(2):
            for i in range(4):
                nc.sync.dma_start(w2sb[n][c][i][:],
                                  w2p[n][c][:, 2304 * i:2304 * (i + 1)])

    rwT_sb = []
    for c in range(2):
        t = cpool.tile([128, 16], F32, tag=f"rwT{c}")
        nc.sync.dma_start(t[:], rwT[c])
        rwT_sb.append(t)
    rb_sb = cpool.tile([16, 1], F32, tag="rb")
    nc.sync.dma_start(rb_sb[:], rb)
    fc1b_sb, bnb_sb = [], []
    fc1w_t = cpool.tile([16, 4096], BF16, tag="fc1w")
    for n in range(2):
        t = cpool.tile([128, 32], F32, tag=f"fc1b{n}")
        if n == 0:
            nc.sync.dma_start(t[:], fc1b[n])
        fc1b_sb.append(t)
        tb = [cpool.tile([128, 1], F32, tag=f"bnb{n}{c}", name=f"bnbt{n}{c}")
              for c in range(2)]
        bnb_sb.append(tb)

    def load_deferred_consts():
        nc.sync.dma_start(fc1b_sb[1][:], fc1b[1])
        for n in range(2):
            for c in range(2):
                nc.sync.dma_start(bnb_sb[n][c][:], bnb[n][c])

    gap16 = cpool.tile([16, BL], BF16, tag="gap16")
    garb = cpool.tile([128, RP * PW], BF16, tag="garb")
    ones_sb = cpool.tile([1, 64], BF16, tag="ones")
    nc.gpsimd.memset(ones_sb[:], 1.0)

    # conv1 output planes (= conv2 input planes): [128, (o,e), RP, PW]
    # zeroed once; sinks only ever write rows 1..56, cols 1..28
    yeo = [cpool.tile([128, 2, RP, PW], BF16, tag=f"yeo{c}", name=f"yeo{c}")
           for c in range(2)]
    for c in range(2):
        nc.gpsimd.memset(yeo[c][:], 0.0)

    # ---- weight generation (emitted as thunks for interleaving) ----
    def gen_weights_a_ops(wn, s, ops):
        """sigmoid(fc1(gap)) -> partition-broadcast coefficient tiles."""
        apsg = psmall_p.tile([128, 33], F32, tag="avec_ps")
        aps = apsg[:, 0:32]
        avt = small_p.tile([128, 32], F32, tag="avtmp")
        avec = small_p.tile([128, 32], BF16, tag="avec")
        avd = dram_p.tile([4096], BF16, tag="avd")
        avlin = avlin_p.tile([1, 4096], BF16, tag="avlin")
        aexp = [aexp_p.tile([128, 4 * 256], BF16, tag=f"aexp{c}",
                            name=f"aexpt{c}") for c in range(2)]
        aps2s = [psmall_p.tile([128, 2 * 256], F32, tag="aexp_ps",
                               name=f"aps2_{half}") for half in range(2)]

        ops.append(lambda: nc.sync.dma_start(fc1w_t[:], fc1wT[wn]))

        def avec_mms():
            for j in range(32):
                nc.tensor.matmul(aps[:, j:j + 1],
                                 fc1w_t[:, 128 * j:128 * (j + 1)],
                                 gap16[:, s:s + 1],
                                 start=True, stop=True)
        ops.append(avec_mms)
        ops.append(lambda: nc.vector.tensor_add(avt[:], aps, fc1b_sb[wn][:]))
        ops.append(lambda: nc.scalar.activation(avec[:], avt[:], AF.Sigmoid))
        ops.append(lambda: nc.sync.dma_start(
            avd[:].rearrange("(j p) -> p j", p=128), avec[:]))
        ops.append(lambda: nc.sync.dma_start(avlin[:], avd[:].unsqueeze(0)))
        avr = avlin[:].rearrange("o (co r) -> o co r", r=16)

        def aexp_mms(c, half):
            def f():
                aps2 = aps2s[half]
                for h in range(2):
                    for ii in range(2):
                        i = 2 * half + ii
                        m = 4 * (2 * c + h) + i
                        rhs = avr[:, :, m:m + 1].rearrange("o co r -> o (co r)")
                        nc.tensor.matmul(
                            aps2[64 * h:64 * (h + 1), 256 * ii:256 * (ii + 1)],
                            ones_sb[:], rhs, start=True, stop=True)
                nc.scalar.copy(aexp[c][:, 512 * half:512 * (half + 1)],
                               aps2[:])
            return f
        for c in range(2):
            for half in range(2):
                ops.append(aexp_mms(c, half))
        return aexp

    def gen_weights_b_ops(wn, aexp, ops):
        """combine 4 basis tensors -> W [128, 9, 256] + 1D U-transform.

        u12[:, kh, 0] = 0.5(W0+W2) + W1h   (wpos1 stationary)
        u12[:, kh, 1] = W1h - 0.5(W0+W2)   (NEGATED wpos2 stationary)
        W1h pre-halved on host.
        """
        res = []
        for c in range(2):
            t = wg_p.tile([128, 9, 256], BF16, tag=f"wg{wn}{c}",
                          name=f"wg{wn}{c}")
            u12 = wg_p.tile([128, 3, 2, 256], BF16, tag=f"u12{wn}{c}",
                            name=f"u12{wn}{c}")

            def abid(i, c=c):
                return (aexp[c][:, 256 * i:256 * (i + 1)].unsqueeze(1)
                        .broadcast_to([128, 9, 256]))

            def k3(ap2d):
                return ap2d.rearrange("p (k co) -> p k co", k=9)

            def mk(c=c, t=t, u12=u12):
                tmps = [wtmp_p.tile([128, 9, 256], BF16, tag="wtmpD",
                                    bufs=1, name=f"wtmp{i}") for i in range(3)]
                yield lambda: nc.vector.tensor_mul(
                    t[:], k3(w2sb[wn][c][0][:]), abid(0, c))
                for i in range(1, 4):
                    tmp = tmps[i - 1]
                    yield lambda i=i, tmp=tmp: nc.vector.tensor_mul(
                        tmp[:], k3(w2sb[wn][c][i][:]), abid(i, c))
                    yield lambda tmp=tmp: nc.vector.tensor_add(
                        t[:], t[:], tmp[:])
                for kh in range(3):
                    uu = wtmp_p.tile([128, 256], BF16, tag="utmp",
                                     name=f"uu{kh}")

                    def uops(kh=kh, uu=uu):
                        nc.vector.tensor_add(uu[:], t[:, 3 * kh, :],
                                             t[:, 3 * kh + 2, :])
                        nc.vector.tensor_scalar_mul(uu[:], uu[:], 0.5)
                        nc.vector.tensor_add(u12[:, kh, 0, :], uu[:],
                                             t[:, 3 * kh + 1, :])
                        nc.vector.tensor_sub(u12[:, kh, 1, :],
                                             t[:, 3 * kh + 1, :], uu[:])
                    yield uops
            ops.extend(mk())
            res.append((t, u12))
        return res

    def gen_weights_ops(wn, s, ops):
        return gen_weights_b_ops(wn, gen_weights_a_ops(wn, s, ops), ops)

    def stat(wu, cc, kh, wpos, occ):
        t, u12 = wu[cc]
        if wpos == 0:
            return t[:, 3 * kh + 0, 128 * occ:128 * occ + 128]
        if wpos == 3:
            return t[:, 3 * kh + 2, 128 * occ:128 * occ + 128]
        return u12[:, kh, wpos - 1, 128 * occ:128 * occ + 128]

    # ---- x loading + gap ----
    def load_x_ops(s, ops):
        planes = []
        for c in range(2):
            te = xeo_p.tile([128, 2, RP, PW], BF16, tag=f"xeo{c}",
                            name=f"xeo{c}")
            planes.append(te)
        gacc = []
        for c in range(2):
            ga = [small_p.tile([128, 1], F32, tag=f"gacc{c}{a}",
                               name=f"gacc{c}{a}") for a in range(2)]
            gs = small_p.tile([128, 1], F32, tag=f"gsum{c}", name=f"gsum{c}")
            gacc.append((ga, gs))
        apsg = psmall_p.tile([128, 33], F32, tag="avec_ps", name="gapps")
        gps = apsg[0:16, 32:33]

        for c in range(2):
            ops.append(lambda c=c: nc.sync.dma_start(planes[c][:], xeo4[s, c]))
        for c in range(2):
            for a in range(2):
                ops.append(lambda c=c, a=a: nc.scalar.activation(
                    garb[:], planes[c][:, a].rearrange("p h w -> p (h w)"),
                    AF.Copy, accum_out=gacc[c][0][a][:]))
            ops.append(lambda c=c: nc.vector.tensor_add(
                gacc[c][1][:], gacc[c][0][0][:], gacc[c][0][1][:]))

        def gapmm():
            for c in range(2):
                nc.tensor.matmul(gps, rwT_sb[c][:], gacc[c][1][:],
                                 start=(c == 0), stop=(c == 1))
            nc.scalar.activation(gap16[:, s:s + 1], gps, AF.Identity,
                                 bias=rb_sb[:], scale=1.0)
        ops.append(gapmm)
        return planes

    # ---- conv: per-t-group V strips + matmuls + sink ----
    # V planes (from o-plane ol/orr, e-plane el/er):
    #   V0 = el - er; V1 = ol + er; V2 = er - ol; V3 = ol - orr
    def conv(wu, planes, sink, bg=None, bg_rate=3):
        for t in range(NT):
            r0 = TR * t
            tws = []
            for c in range(2):
                st = twS_p.tile([128, 4, TR + 2, TC], BF16, tag=f"tws{c}",
                                name=f"tws{c}")
                e = planes[c][:, 1, r0:r0 + TR + 2, :]
                o = planes[c][:, 0, r0:r0 + TR + 2, :]
                el = e[:, :, 0:TC]
                er = e[:, :, 1:TC + 1]
                ol = o[:, :, 1:TC + 1]
                orr = o[:, :, 2:TC + 2]
                nc.vector.tensor_sub(st[:, 0], el, er)
                nc.vector.tensor_add(st[:, 1], ol, er)
                nc.gpsimd.tensor_sub(st[:, 2], er, ol)
                nc.gpsimd.tensor_sub(st[:, 3], ol, orr)
                tws.append(st)
            if bg:
                for _ in range(bg_rate):
                    if bg:
                        bg.popleft()()
            for occ in range(2):
                ps = psum_p.tile([128, 4, 256], F32, tag="cps")
                for wpos in range(4):
                    for cc in range(2):
                        for kh in range(3):
                            nc.tensor.matmul(
                                ps[:, wpos, 0:NG],
                                stat(wu, cc, kh, wpos, occ),
                                tws[cc][:, wpos, kh:kh + TR, :],
                                start=(cc == 0 and kh == 0),
                                stop=(cc == 1 and kh == 2))
                sink(occ, t, ps)

    # Y-stage: with M2' = -M2 (negated U2 stationary):
    #   [e1,o1] = m[(0,2)] + bcast(m1);  [e2,o2] = [e1,o1] - m[(2,3)]
    #   e2 = M0+M1+M2 (even outputs), o2 = M1-M2-M3 (odd outputs)
    def ystage(ps):
        m = stage_p.tile([128, 4, NG], BF16, tag="mev")
        nc.scalar.copy(m[:], ps[:, :, 0:NG])
        m02 = m[:].rearrange("p (j two) n -> p j two n", two=2)[:, :, 0]
        eo1 = stage_p.tile([128, 2, NG], BF16, tag="eo1", bufs=1)
        nc.vector.tensor_add(
            eo1[:], m02, m[:, 1].unsqueeze(1).broadcast_to([128, 2, NG]))
        eo2 = stage_p.tile([128, 2, NG], BF16, tag="eo2")
        nc.vector.tensor_sub(eo2[:], eo1[:], m[:, 2:4])
        return eo2

    def sink1(occ, t, ps):
        r0 = TR * t + 1
        eo2 = ystage(ps)
        # even outputs -> o-plane, odd outputs -> e-plane, cols 1..28
        nc.scalar.activation(
            yeo[occ][:, :, r0:r0 + TR, 1:TC + 1],
            eo2[:].rearrange("p a (h w) -> p a h w", h=TR),
            AF.Relu, bias=bnb_sb[0][occ][:], scale=1.0)

    def make_sink2(s, xplanes):
        def sink2(occ, t, ps):
            r0 = TR * t + 1
            eo2 = ystage(ps)
            rx = xplanes[occ][:, :, r0:r0 + TR, 1:TC + 1]
            eo3 = stage_p.tile([128, 2, TR, TC], BF16, tag="eo3")
            nc.vector.tensor_add(
                eo3[:], eo2[:].rearrange("p a (h w) -> p a h w", h=TR), rx)
            os_ = stage_p.tile([128, 2, TR, TC], BF16, tag="ostg")
            nc.scalar.activation(os_[:], eo3[:], AF.Relu,
                                 bias=bnb_sb[1][occ][:], scale=1.0)
            nc.sync.dma_start(outd[s, occ][:, :, TR * t:TR * t + TR, :],
                              os_[:])
        return sink2

    # ---- main pipeline ----
    from collections import deque
    ops0 = []
    xp = load_x_ops(0, ops0)
    for f in ops0:
        f()
    ops0 = []
    w1 = gen_weights_ops(0, 0, ops0)
    load_w2sb(0)
    for f in ops0:
        f()
    load_deferred_consts()
    load_w2sb(1)

    for s in range(BL):
        bg1 = deque()
        w2 = gen_weights_ops(1, s, bg1)
        bg2 = deque()
        if s + 1 < BL:
            xp_n = load_x_ops(s + 1, bg2)
            w1_n = gen_weights_ops(0, s + 1, bg2)

        conv(w1, xp, sink1, bg=bg1)
        while bg1:
            bg1.popleft()()
        conv(w2, yeo, make_sink2(s, xp), bg=bg2)
        while bg2:
            bg2.popleft()()

        if s + 1 < BL:
            xp, w1 = xp_n, w1_n

    ctx.close()


_NC_CACHE = {}


def get_program():
    if "nc" not in _NC_CACHE:
        _NC_CACHE["nc"] = build_program()
    return _NC_CACHE["nc"]


def prep_inputs(inputs):
    x = np.asarray(inputs["x"], np.float32)
    f32 = lambda a: np.ascontiguousarray(np.asarray(a, np.float32))
    bf = lambda a: np.ascontiguousarray(
        np.asarray(a, np.float32).astype(ml_dtypes.bfloat16))

    def bn_fold(g, b, m, v):
        sc = np.asarray(g, np.float32) / np.sqrt(np.asarray(v, np.float32) + EPS)
        bia = np.asarray(b, np.float32) - np.asarray(m, np.float32) * sc
        return sc, f32(bia.reshape(2, 128, 1))

    def pack_w2(fc2_w, bn_sc):
        w2_ = np.asarray(fc2_w, np.float32).reshape(256, 4, 64, 9, 4)
        w2_ = w2_ * bn_sc[:, None, None, None, None]   # fold bn scale (per oc)
        w2_[:, :, :, 1::3, :] *= 0.5                   # pre-halve kw=1 taps
        w2h = w2_.transpose(4, 3, 1, 2, 0).reshape(4, 9, 256, 256)
        return bf(w2h.transpose(2, 0, 1, 3).reshape(2, 128, 4 * 9 * 256))

    s1, b1 = bn_fold(inputs["bn1_g"], inputs["bn1_b"],
                     inputs["bn1_m"], inputs["bn1_v"])
    s2, b2 = bn_fold(inputs["bn2_g"], inputs["bn2_b"],
                     inputs["bn2_m"], inputs["bn2_v"])

    NPIX = H * W
    base = {
        "rwT": f32((np.asarray(inputs["reduce_w"], np.float32).T / NPIX)
                   .reshape(2, 128, 16)),
        "rb": f32(np.asarray(inputs["reduce_b"]).reshape(16, 1)),
        "fc1wT1": bf(np.asarray(inputs["w1_fc1_w"]).T),
        "fc1wT2": bf(np.asarray(inputs["w2_fc1_w"]).T),
        "fc1b1": f32(np.asarray(inputs["w1_fc1_b"]).reshape(32, 128).T),
        "fc1b2": f32(np.asarray(inputs["w2_fc1_b"]).reshape(32, 128).T),
        "w2p1": pack_w2(inputs["w1_fc2_w"], s1),
        "w2p2": pack_w2(inputs["w2_fc2_w"], s2),
        "bnb1": b1,
        "bnb2": b2,
    }

    # host-prepadded planes: o-plane[j] = xpad[2j-1] (x even cols, at 1..28),
    # e-plane[j] = xpad[2j] (x odd cols at 1..28; col 0 = xpad[0] = 0)
    xb = x.astype(ml_dtypes.bfloat16)
    xeo = np.zeros((B, C, 2, RP, PW), ml_dtypes.bfloat16)
    xeo[:, :, 0, 1:RP - 1, 1:TC + 1] = xb[:, :, :, 0::2]
    xeo[:, :, 1, 1:RP - 1, 1:TC + 1] = xb[:, :, :, 1::2]

    in_maps = []
    for i in range(NCORES):
        m = dict(base)
        m["xeo4"] = np.ascontiguousarray(
            xeo[i * BL:(i + 1) * BL].reshape(BL, 2, 128, 2, RP, PW))
        in_maps.append(m)
    return in_maps


def unpack_outputs(results):
    outs = []
    for r in results:
        od = np.asarray(r["outd"], ml_dtypes.bfloat16).astype(np.float32)
        out = np.zeros((BL, 2, 128, H, W), np.float32)
        out[..., 0::2] = od[:, :, :, 0]
        out[..., 1::2] = od[:, :, :, 1]
        outs.append(out.reshape(BL, C, H, W))
    return np.concatenate(outs, axis=0)


def kernel(**inputs):
    in_maps = prep_inputs(inputs)
    nc = get_program()
    res = bass_utils.run_bass_kernel_spmd(nc, in_maps,
                                          core_ids=list(range(NCORES)))
    return unpack_outputs(res.results)
